# revision 2
# baseline (speedup 1.0000x reference)
"""DenseWarp (bilinear dense_image_warp) Bass kernel for 8 axon trn2 cores.

Sharding: core i -> batch b = i//2, row-half = i%2 (540 of 1080 rows).

Per-core pipeline ("pairwarp"):
  phase A (DVE, row layout [P,1920]): wy/wx = frac(clip(y-fy)), bf16.
  phase B (DVE, gather-wrapped layout): u = y0rel*254 + x0 (global), i16
      per-chunk rebased.  Wrapped layout = ap_gather idx order, so the idx
      tile IS the phase-B output (no swizzle spill).
  per set (supertile x 240-col chunk): load host-prepped bf16 pair tables
      (unit u holds (F[y,x], F[y,x+1]); partitions 16g+4s+ch hold channel
      ch shifted down s rows), ONE ap_gather returns all 4 corners,
      8 stepped-partition DMAs rearrange gather streams to row layout,
      DVE lerps with natural-layout weights, bf16 out assembled per
      supertile and stored (host converts to f32).
"""
import sys
import numpy as np

sys.path.insert(0, '/opt/trn_rl_repo')

import ml_dtypes
from concourse import bass, bacc, tile
from concourse.bass import mybir
from concourse.bass_utils import run_bass_kernel_spmd

f32 = mybir.dt.float32
bf16 = mybir.dt.bfloat16
i16 = mybir.dt.int16
i32 = mybir.dt.int32

B, C, H, W = 4, 4, 1080, 1920
HALF = H // 2              # 540
XC = 240                   # chunk cols
NCHUNK = W // XC           # 8
PATW = 254                 # pair-table row width (units)
PATR = 28                  # y0rel in [0,27]
NU = PATR * PATW           # 7112 units (each 2 bf16 = 4B)
RT = 16                    # rowtile rows
NIDX = RT * XC             # 3840 idx per group
NT = 34                    # rowtiles per half
ROWBASES = [RT * i for i in range(33)] + [524]
# supertiles: (first rowtile, n rowtiles)
SUPS = [(0, 8), (8, 8), (16, 8), (24, 8), (32, 2)]
NSUP = len(SUPS)

AddOp = mybir.AluOpType.add
SubOp = mybir.AluOpType.subtract
MulOp = mybir.AluOpType.mult
MaxOp = mybir.AluOpType.max
MinOp = mybir.AluOpType.min
GtOp = mybir.AluOpType.is_gt


def build():
    nc = bacc.Bacc("TRN2", target_bir_lowering=False, debug=False,
                   num_devices=8)

    tbl_d = nc.dram_tensor("tbl_d", [NT, NCHUNK, 8, NU * 2], bf16,
                           kind="ExternalInput").ap()
    flr_d = nc.dram_tensor("flr_d", [2, NSUP, 128, W], f32,
                           kind="ExternalInput").ap()
    flw_d = nc.dram_tensor("flw_d", [2, NSUP, 128, W], f32,
                           kind="ExternalInput").ap()
    yww_d = nc.dram_tensor("yww_d", [NSUP, 128, W], f32,
                           kind="ExternalInput").ap()
    ygr_d = nc.dram_tensor("ygr_d", [NSUP, 128], f32,
                           kind="ExternalInput").ap()
    rb6_d = nc.dram_tensor("rb6_d", [NSUP, 128], f32,
                           kind="ExternalInput").ap()
    xww_d = nc.dram_tensor("xww_d", [16, W], f32, kind="ExternalInput").ap()
    xgn_d = nc.dram_tensor("xgn_d", [W], f32, kind="ExternalInput").ap()
    out_d = nc.dram_tensor("out_d", [C, HALF, W], bf16,
                           kind="ExternalOutput").ap()

    with tile.TileContext(nc) as tc:
        with tc.tile_pool(name="pc", bufs=1) as pc, \
             tc.tile_pool(name="ps", bufs=2) as ps, \
             tc.tile_pool(name="pf", bufs=1) as pf, \
             tc.tile_pool(name="pt", bufs=2) as pt, \
             tc.tile_pool(name="pq", bufs=1) as pq:
            # resident consts
            xw_t = pc.tile([128, W], f32, name="xw_t")
            xg_t = pc.tile([128, W], f32, name="xg_t")
            nc.sync.dma_start(
                xw_t[:], bass.AP(xww_d.tensor, 0, [[0, 8], [W, 16], [1, W]]))
            nc.sync.dma_start(
                xg_t[:], bass.AP(xgn_d.tensor, 0, [[0, 128], [1, W]]))

            def emit_phase(sup):
                t0, ng = SUPS[sup]
                P = 16 * ng
                sl = slice(0, P)
                wy = ps.tile([128, W], bf16, name=f"wy{sup}", tag="wy")
                wx = ps.tile([128, W], bf16, name=f"wx{sup}", tag="wx")
                u16 = ps.tile([128, W], i16, name=f"u16_{sup}", tag="u16")
                yg = ps.tile([128, 1], f32, name=f"yg{sup}", tag="yg")
                rb6 = ps.tile([128, 1], f32, name=f"rb6_{sup}", tag="rb6")
                nc.sync.dma_start(yg[sl], ygr_d[sup, :P].unsqueeze(1))
                nc.sync.dma_start(rb6[sl], rb6_d[sup, :P].unsqueeze(1))
                F1 = pf.tile([128, W], f32, name=f"F1_{sup}", tag="F1")
                F2 = pf.tile([128, W], f32, name=f"F2_{sup}", tag="F2")
                F3 = pf.tile([128, W], f32, name=f"F3_{sup}", tag="F3")
                F4 = pf.tile([128, W], f32, name=f"F4_{sup}", tag="F4")
                It = pf.tile([128, W], i32, name=f"It_{sup}", tag="It")

                def floor_into(q, dst, scr):
                    # dst = floor(q); clobbers It, dst, scr
                    nc.vector.tensor_copy(It[sl], q)
                    nc.vector.tensor_copy(dst, It[sl])
                    nc.vector.tensor_tensor(scr, dst, q, GtOp)
                    nc.vector.tensor_sub(dst, dst, scr)

                # ---- phase A (row layout) ----
                nc.sync.dma_start(F1[sl], flr_d[0, sup, :P])
                nc.sync.dma_start(F2[sl], flr_d[1, sup, :P])
                # qy = clip(y - fy) = clip((fy - yg)*-1)
                nc.vector.tensor_scalar(F3[sl], F1[sl], yg[sl], -1.0,
                                        op0=SubOp, op1=MulOp)
                nc.vector.tensor_scalar(F3[sl], F3[sl], 0.0, float(H - 1),
                                        op0=MaxOp, op1=MinOp)
                floor_into(F3[sl], F4[sl], F1[sl])
                nc.vector.tensor_sub(wy[sl], F3[sl], F4[sl])
                # qx = clip(xg - fx)
                nc.vector.tensor_sub(F3[sl], xg_t[sl], F2[sl])
                nc.vector.tensor_scalar(F3[sl], F3[sl], 0.0, float(W - 1),
                                        op0=MaxOp, op1=MinOp)
                floor_into(F3[sl], F4[sl], F1[sl])
                nc.vector.tensor_sub(wx[sl], F3[sl], F4[sl])

                # ---- phase B (wrapped layout) ----
                nc.sync.dma_start(F1[sl], flw_d[0, sup, :P])
                nc.sync.dma_start(F2[sl], flw_d[1, sup, :P])
                nc.sync.dma_start(F3[sl], yww_d[sup, :P])
                # qy = clip((fyw - yw)*-1)  (bit-identical to phase A)
                nc.vector.tensor_tensor(F3[sl], F1[sl], F3[sl], SubOp)
                nc.vector.tensor_scalar(F3[sl], F3[sl], -1.0, 0.0,
                                        op0=MulOp, op1=AddOp)
                nc.vector.tensor_scalar(F3[sl], F3[sl], 0.0, float(H - 1),
                                        op0=MaxOp, op1=MinOp)
                floor_into(F3[sl], F4[sl], F1[sl])
                # uy = (y0f - (rowbase-6)) * 254
                nc.vector.tensor_scalar(F4[sl], F4[sl], rb6[sl], float(PATW),
                                        op0=SubOp, op1=MulOp)
                # qx = clip(xw - fxw)
                nc.vector.tensor_sub(F3[sl], xw_t[sl], F2[sl])
                nc.vector.tensor_scalar(F3[sl], F3[sl], 0.0, float(W - 1),
                                        op0=MaxOp, op1=MinOp)
                floor_into(F3[sl], F1[sl], F2[sl])
                # uf = uy + x0f (global x; chunk rebase at i16 convert)
                nc.vector.tensor_add(F4[sl], F4[sl], F1[sl])
                for k in range(NCHUNK):
                    ck = slice(XC * k, XC * (k + 1))
                    nc.vector.tensor_scalar(F2[sl, ck], F4[sl, ck],
                                            float(6 - XC * k), 0.0,
                                            op0=AddOp, op1=AddOp)
                    nc.vector.tensor_copy(u16[sl, ck], F2[sl, ck])
                return wy, wx, u16

            state = {0: emit_phase(0)}
            for sup in range(NSUP):
                t0, ng = SUPS[sup]
                P = 16 * ng
                sl = slice(0, P)
                if sup + 1 < NSUP:
                    state[sup + 1] = emit_phase(sup + 1)
                wy, wx, u16 = state.pop(sup)
                outts = [pq.tile([128, W], bf16, name=f"o{sup}_{ch}",
                                 tag=f"outt{ch}") for ch in range(C)]
                qrow = None
                for k in range(NCHUNK):
                    tbl = pt.tile([128, NU * 2], bf16, name=f"tb{sup}_{k}",
                                  tag="tbl")
                    for g in range(ng):
                        t = t0 + g
                        nc.sync.dma_start(tbl[16 * g:16 * g + 8, :],
                                          tbl_d[t, k])
                    gout = pt.tile([128, NIDX * 2], bf16,
                                   name=f"go{sup}_{k}", tag="gout")
                    nc.gpsimd.ap_gather(
                        gout[sl], tbl[sl], u16[sl, XC * k:XC * (k + 1)],
                        channels=P, num_elems=NU, d=2, num_idxs=NIDX)
                    kb = k % 2
                    if kb == 0:
                        qrow = pq.tile([128, 2 * 8 * XC * 2], bf16,
                                       name=f"qr{sup}_{k}", tag="qrow")
                    for pl in range(8):
                        src = gout[pl:pl + 16 * (ng - 1) + 1:16, :].rearrange(
                            'p (r e) -> p r e', r=RT)
                        blk = (kb * 8 + pl) * XC * 2
                        nc.scalar.dma_start(qrow[sl, blk:blk + XC * 2], src)
                    if kb == 1:
                        # lerp 2-chunk batch [P, 2, 240]
                        cs = slice(XC * (k - 1), XC * (k + 1))
                        q5 = qrow[sl].rearrange(
                            'p (kb pl c e) -> p kb pl c e', kb=2, pl=8, e=2)
                        WX = wx[sl, cs].rearrange('p (a b) -> p a b', b=XC)
                        WY = wy[sl, cs].rearrange('p (a b) -> p a b', b=XC)
                        for ch in range(C):
                            q00 = q5[:, :, ch, :, 0]
                            q01 = q5[:, :, ch, :, 1]
                            q10 = q5[:, :, 4 + ch, :, 0]
                            q11 = q5[:, :, 4 + ch, :, 1]
                            Ts = pq.tile([128, 2 * XC], bf16,
                                         name=f"T{sup}_{k}_{ch}", tag="Ts")
                            Av = pq.tile([128, 2 * XC], bf16,
                                         name=f"A{sup}_{k}_{ch}", tag="Av")
                            Bv = pq.tile([128, 2 * XC], bf16,
                                         name=f"B{sup}_{k}_{ch}", tag="Bv")
                            A3 = Av[sl].rearrange('p (a b) -> p a b', b=XC)
                            B3 = Bv[sl].rearrange('p (a b) -> p a b', b=XC)
                            T3 = Ts[sl].rearrange('p (a b) -> p a b', b=XC)
                            O3 = outts[ch][sl, cs].rearrange(
                                'p (a b) -> p a b', b=XC)
                            nc.vector.tensor_sub(T3, q01, q00)
                            nc.vector.tensor_mul(T3, T3, WX)
                            nc.vector.tensor_add(A3, q00, T3)
                            nc.vector.tensor_sub(T3, q11, q10)
                            nc.vector.tensor_mul(T3, T3, WX)
                            nc.vector.tensor_add(B3, q10, T3)
                            nc.vector.tensor_sub(T3, B3, A3)
                            nc.vector.tensor_mul(T3, T3, WY)
                            nc.vector.tensor_add(O3, A3, T3)
                # store supertile output
                for ch in range(C):
                    if sup < 4:
                        r0 = ROWBASES[t0]
                        nc.scalar.dma_start(
                            out_d[ch, r0:r0 + 128, :], outts[ch][0:128, :])
                    else:
                        nc.scalar.dma_start(
                            out_d[ch, 512:528, :], outts[ch][0:16, :])
                        nc.scalar.dma_start(
                            out_d[ch, 524:540, :], outts[ch][16:32, :])

    nc.compile()
    return nc


_cache = {}


def _get_nc():
    if 'nc' not in _cache:
        _cache['nc'] = build()
    return _cache['nc']


def _host_inputs(frame, flow):
    frame = np.ascontiguousarray(frame, dtype=np.float32)
    flow = np.ascontiguousarray(flow, dtype=np.float32)
    fbf = frame.astype(ml_dtypes.bfloat16)

    # wrapped free-order maps: o -> (chunk, r, j); col = 240*chunk + 16*j + q
    oo = np.arange(W)
    kk = oo // XC
    rr = (oo % XC) // 15
    jj = (oo % XC) % 15
    colbase = XC * kk + 16 * jj                      # [W]
    rowbases = np.array(ROWBASES, np.int64)

    xww = (colbase[None, :] + np.arange(16)[:, None]).astype(np.float32)
    xgn = np.arange(W, dtype=np.float32)

    # per-sup partition -> rowtile map
    g_of_p = np.arange(128) // 16                    # [128]
    q_of_p = np.arange(128) % 16

    # table row/col gather indices (global, clipped)
    colidx = np.clip(np.arange(NCHUNK)[:, None] * XC - 6
                     + np.arange(PATW + 1)[None, :], 0, W - 1)  # [8,255]

    in_maps = []
    for core in range(8):
        b, half = divmod(core, 2)
        # ---- tables ----
        # rows[t, s, y] global
        rowidx = np.clip(half * HALF + rowbases[:, None, None] - 6
                         + np.arange(2)[None, :, None]
                         + np.arange(PATR + 1)[None, None, :], 0, H - 1)
        # patches [C, NT, 2, 29, 8, 255]
        patches = fbf[b][:, rowidx[:, :, :, None, None],
                         colidx[None, None, None, :, :]]
        pair = np.stack([patches[..., :PATR, :, :PATW],
                         patches[..., :PATR, :, 1:PATW + 1]], axis=-1)
        # -> [NT, chunk, (s,c), y, x, e] -> [NT, 8, 8, NU*2]
        tbl = np.ascontiguousarray(
            pair.transpose(1, 4, 2, 0, 3, 5, 6)).reshape(NT, NCHUNK, 8,
                                                         NU * 2)

        flr = np.zeros((2, NSUP, 128, W), np.float32)
        flw = np.zeros((2, NSUP, 128, W), np.float32)
        yww = np.zeros((NSUP, 128, W), np.float32)
        ygr = np.zeros((NSUP, 128), np.float32)
        rb6 = np.zeros((NSUP, 128), np.float32)
        fl = flow[b]                                  # [2, 1080, 1920]
        for sup, (t0, ng) in enumerate(SUPS):
            P = 16 * ng
            grow = half * HALF + rowbases[t0 + g_of_p[:P]]   # rowbase
            # row layout: partition p -> row rowbase + q? NO: row layout
            # partition p of supertile = rowbase(t0 + p//16) + p%16
            rowsA = grow + q_of_p[:P]
            flr[:, sup, :P, :] = fl[:, rowsA, :]
            ygr[sup, :P] = rowsA
            rb6[sup, :P] = grow - 6
            # wrapped: partition 16g+q holds pixels (rowbase+rr[o],
            #          colbase[o]+q)
            rowm = grow[:, None] + rr[None, :]               # [P, W]
            colm = colbase[None, :] + q_of_p[:P, None]       # [P, W]
            flw[:, sup, :P, :] = fl[:, rowm, colm]
            yww[sup, :P, :] = rowm.astype(np.float32)
        in_maps.append({
            "tbl_d": tbl,
            "flr_d": flr,
            "flw_d": flw,
            "yww_d": yww,
            "ygr_d": ygr,
            "rb6_d": rb6,
            "xww_d": xww,
            "xgn_d": xgn,
        })
    return in_maps


def run(frame, flow, trace=False, tmpdir=None):
    nc = _get_nc()
    in_maps = _host_inputs(frame, flow)
    res = run_bass_kernel_spmd(nc, in_maps, core_ids=list(range(8)),
                               trace=trace, tmpdir=tmpdir)
    out = np.empty((B, C, H, W), np.float32)
    for core in range(8):
        b, half = divmod(core, 2)
        out[b, :, half * HALF:(half + 1) * HALF, :] = \
            res.results[core]["out_d"].astype(np.float32)
    return out, res


def kernel(frame, flow):
    out, _ = run(np.asarray(frame), np.asarray(flow))
    return out


# revision 5
# speedup vs baseline: 1.0073x; 1.0073x over previous
"""DenseWarp (bilinear dense_image_warp) Bass kernel for 8 axon trn2 cores.

Sharding: core i -> batch b = i//2, row-half = i%2 (540 of 1080 rows).

Per-core pipeline ("pairwarp"):
  phase A (DVE, row layout [P,1920]): wy/wx = frac(clip(y-fy)), bf16.
  phase B (DVE, gather-wrapped layout): u = y0rel*254 + x0 (global), i16
      per-chunk rebased.  Wrapped layout = ap_gather idx order, so the idx
      tile IS the phase-B output (no swizzle spill).
  per set (supertile x 240-col chunk): load host-prepped bf16 pair tables
      (unit u holds (F[y,x], F[y,x+1]); partitions 16g+4s+ch hold channel
      ch shifted down s rows), ONE ap_gather returns all 4 corners,
      8 stepped-partition DMAs rearrange gather streams to row layout,
      DVE lerps with natural-layout weights, bf16 out assembled per
      supertile and stored (host converts to f32).
"""
import sys
import numpy as np

sys.path.insert(0, '/opt/trn_rl_repo')

import ml_dtypes
from concourse import bass, bacc, tile
from concourse.bass import mybir
from concourse.bass_utils import run_bass_kernel_spmd

f32 = mybir.dt.float32
bf16 = mybir.dt.bfloat16
i16 = mybir.dt.int16
i32 = mybir.dt.int32

B, C, H, W = 4, 4, 1080, 1920
HALF = H // 2              # 540
XC = 240                   # chunk cols
NCHUNK = W // XC           # 8
PATW = 254                 # pair-table row width (units)
PATR = 28                  # y0rel in [0,27]
NU = PATR * PATW           # 7112 units (each 2 bf16 = 4B)
RT = 16                    # rowtile rows
NIDX = RT * XC             # 3840 idx per group
NT = 34                    # rowtiles per half
ROWBASES = [RT * i for i in range(33)] + [524]
# supertiles: (first rowtile, n rowtiles)
SUPS = [(0, 8), (8, 8), (16, 8), (24, 8), (32, 2)]
NSUP = len(SUPS)

AddOp = mybir.AluOpType.add
SubOp = mybir.AluOpType.subtract
MulOp = mybir.AluOpType.mult
MaxOp = mybir.AluOpType.max
MinOp = mybir.AluOpType.min
GtOp = mybir.AluOpType.is_gt


def build():
    nc = bacc.Bacc("TRN2", target_bir_lowering=False, debug=False,
                   num_devices=8)

    tbl_d = nc.dram_tensor("tbl_d", [NT, NCHUNK, 8, NU * 2], bf16,
                           kind="ExternalInput").ap()
    flr_d = nc.dram_tensor("flr_d", [2, NSUP, 128, W], f32,
                           kind="ExternalInput").ap()
    flw_d = nc.dram_tensor("flw_d", [2, NSUP, 128, W], f32,
                           kind="ExternalInput").ap()
    yww_d = nc.dram_tensor("yww_d", [NSUP, 128, W], f32,
                           kind="ExternalInput").ap()
    ygr_d = nc.dram_tensor("ygr_d", [NSUP, 128], f32,
                           kind="ExternalInput").ap()
    rb6_d = nc.dram_tensor("rb6_d", [NSUP, 128], f32,
                           kind="ExternalInput").ap()
    xww_d = nc.dram_tensor("xww_d", [16, W], f32, kind="ExternalInput").ap()
    xgn_d = nc.dram_tensor("xgn_d", [W], f32, kind="ExternalInput").ap()
    out_d = nc.dram_tensor("out_d", [C, HALF, W], bf16,
                           kind="ExternalOutput").ap()

    with tile.TileContext(nc) as tc:
        with tc.tile_pool(name="pc", bufs=1) as pc, \
             tc.tile_pool(name="ps", bufs=2) as ps, \
             tc.tile_pool(name="pf", bufs=1) as pf, \
             tc.tile_pool(name="pt", bufs=2) as pt, \
             tc.tile_pool(name="pq", bufs=1) as pq:
            # resident consts
            xw_t = pc.tile([128, W], f32, name="xw_t")
            xg_t = pc.tile([128, W], f32, name="xg_t")
            nc.sync.dma_start(
                xw_t[:], bass.AP(xww_d.tensor, 0, [[0, 8], [W, 16], [1, W]]))
            nc.sync.dma_start(
                xg_t[:], bass.AP(xgn_d.tensor, 0, [[0, 128], [1, W]]))

            def emit_phase(sup):
                t0, ng = SUPS[sup]
                P = 16 * ng
                sl = slice(0, P)
                wy = ps.tile([128, W], bf16, name=f"wy{sup}", tag="wy")
                wx = ps.tile([128, W], bf16, name=f"wx{sup}", tag="wx")
                u16 = ps.tile([128, W], i16, name=f"u16_{sup}", tag="u16")
                yg = ps.tile([128, 1], f32, name=f"yg{sup}", tag="yg")
                rb6 = ps.tile([128, 1], f32, name=f"rb6_{sup}", tag="rb6")
                nc.scalar.dma_start(yg[sl], ygr_d[sup, :P].unsqueeze(1))
                nc.scalar.dma_start(rb6[sl], rb6_d[sup, :P].unsqueeze(1))
                F1 = pf.tile([128, W], f32, name=f"F1_{sup}", tag="F1")
                F2 = pf.tile([128, W], f32, name=f"F2_{sup}", tag="F2")
                F3 = pf.tile([128, W], f32, name=f"F3_{sup}", tag="F3")
                F4 = pf.tile([128, W], f32, name=f"F4_{sup}", tag="F4")
                It = pf.tile([128, W], i32, name=f"It_{sup}", tag="It")

                def floor_into(q, dst, scr):
                    # dst = floor(q); clobbers It, dst, scr
                    nc.vector.tensor_copy(It[sl], q)
                    nc.vector.tensor_copy(dst, It[sl])
                    nc.vector.tensor_tensor(scr, dst, q, GtOp)
                    nc.vector.tensor_sub(dst, dst, scr)

                # ---- phase A (row layout) ----
                nc.scalar.dma_start(F1[sl], flr_d[0, sup, :P])
                nc.scalar.dma_start(F2[sl], flr_d[1, sup, :P])
                # qy = clip(y - fy) = clip((fy - yg)*-1)
                nc.vector.tensor_scalar(F3[sl], F1[sl], yg[sl], -1.0,
                                        op0=SubOp, op1=MulOp)
                nc.vector.tensor_scalar(F3[sl], F3[sl], 0.0, float(H - 1),
                                        op0=MaxOp, op1=MinOp)
                floor_into(F3[sl], F4[sl], F1[sl])
                nc.vector.tensor_sub(wy[sl], F3[sl], F4[sl])
                # qx = clip(xg - fx)
                nc.vector.tensor_sub(F3[sl], xg_t[sl], F2[sl])
                nc.vector.tensor_scalar(F3[sl], F3[sl], 0.0, float(W - 1),
                                        op0=MaxOp, op1=MinOp)
                floor_into(F3[sl], F4[sl], F1[sl])
                nc.vector.tensor_sub(wx[sl], F3[sl], F4[sl])

                # ---- phase B (wrapped layout) ----
                nc.scalar.dma_start(F1[sl], flw_d[0, sup, :P])
                nc.scalar.dma_start(F2[sl], flw_d[1, sup, :P])
                nc.scalar.dma_start(F3[sl], yww_d[sup, :P])
                # qy = clip((fyw - yw)*-1)  (bit-identical to phase A)
                nc.vector.tensor_tensor(F3[sl], F1[sl], F3[sl], SubOp)
                nc.vector.tensor_scalar(F3[sl], F3[sl], -1.0, 0.0,
                                        op0=MulOp, op1=AddOp)
                nc.vector.tensor_scalar(F3[sl], F3[sl], 0.0, float(H - 1),
                                        op0=MaxOp, op1=MinOp)
                floor_into(F3[sl], F4[sl], F1[sl])
                # uy = (y0f - (rowbase-6)) * 254
                nc.vector.tensor_scalar(F4[sl], F4[sl], rb6[sl], float(PATW),
                                        op0=SubOp, op1=MulOp)
                # qx = clip(xw - fxw)
                nc.vector.tensor_sub(F3[sl], xw_t[sl], F2[sl])
                nc.vector.tensor_scalar(F3[sl], F3[sl], 0.0, float(W - 1),
                                        op0=MaxOp, op1=MinOp)
                floor_into(F3[sl], F1[sl], F2[sl])
                # uf = uy + x0f (global x; chunk rebase at i16 convert)
                nc.vector.tensor_add(F4[sl], F4[sl], F1[sl])
                for k in range(NCHUNK):
                    ck = slice(XC * k, XC * (k + 1))
                    nc.vector.tensor_scalar(F2[sl, ck], F4[sl, ck],
                                            float(6 - XC * k), 0.0,
                                            op0=AddOp, op1=AddOp)
                    nc.vector.tensor_copy(u16[sl, ck], F2[sl, ck])
                return wy, wx, u16

            state = {0: emit_phase(0)}
            for sup in range(NSUP):
                t0, ng = SUPS[sup]
                P = 16 * ng
                sl = slice(0, P)
                if sup + 1 < NSUP:
                    state[sup + 1] = emit_phase(sup + 1)
                wy, wx, u16 = state.pop(sup)
                outts = [pq.tile([128, W], bf16, name=f"o{sup}_{ch}",
                                 tag=f"outt{ch}") for ch in range(C)]
                qrow = None
                for k in range(NCHUNK):
                    tbl = pt.tile([128, NU * 2], bf16, name=f"tb{sup}_{k}",
                                  tag="tbl")
                    for g in range(ng):
                        t = t0 + g
                        nc.sync.dma_start(tbl[16 * g:16 * g + 8, :],
                                          tbl_d[t, k])
                    gout = pt.tile([128, NIDX * 2], bf16,
                                   name=f"go{sup}_{k}", tag="gout")
                    nc.gpsimd.ap_gather(
                        gout[sl], tbl[sl], u16[sl, XC * k:XC * (k + 1)],
                        channels=P, num_elems=NU, d=2, num_idxs=NIDX)
                    kb = k % 2
                    if kb == 0:
                        qrow = pq.tile([128, 2 * 8 * XC * 2], bf16,
                                       name=f"qr{sup}_{k}", tag="qrow")
                    for pl in range(8):
                        src = gout[pl:pl + 16 * (ng - 1) + 1:16, :].rearrange(
                            'p (r e) -> p r e', r=RT)
                        blk = (kb * 8 + pl) * XC * 2
                        nc.scalar.dma_start(qrow[sl, blk:blk + XC * 2], src)
                    if kb == 1:
                        # lerp 2-chunk batch [P, 2, 240]
                        cs = slice(XC * (k - 1), XC * (k + 1))
                        q5 = qrow[sl].rearrange(
                            'p (kb pl c e) -> p kb pl c e', kb=2, pl=8, e=2)
                        WX = wx[sl, cs].rearrange('p (a b) -> p a b', b=XC)
                        WY = wy[sl, cs].rearrange('p (a b) -> p a b', b=XC)
                        for ch in range(C):
                            q00 = q5[:, :, ch, :, 0]
                            q01 = q5[:, :, ch, :, 1]
                            q10 = q5[:, :, 4 + ch, :, 0]
                            q11 = q5[:, :, 4 + ch, :, 1]
                            Ts = pq.tile([128, 2 * XC], bf16,
                                         name=f"T{sup}_{k}_{ch}", tag="Ts")
                            Av = pq.tile([128, 2 * XC], bf16,
                                         name=f"A{sup}_{k}_{ch}", tag="Av")
                            Bv = pq.tile([128, 2 * XC], bf16,
                                         name=f"B{sup}_{k}_{ch}", tag="Bv")
                            A3 = Av[sl].rearrange('p (a b) -> p a b', b=XC)
                            B3 = Bv[sl].rearrange('p (a b) -> p a b', b=XC)
                            T3 = Ts[sl].rearrange('p (a b) -> p a b', b=XC)
                            O3 = outts[ch][sl, cs].rearrange(
                                'p (a b) -> p a b', b=XC)
                            nc.vector.tensor_sub(T3, q01, q00)
                            nc.vector.tensor_mul(T3, T3, WX)
                            nc.vector.tensor_add(A3, q00, T3)
                            nc.vector.tensor_sub(T3, q11, q10)
                            nc.vector.tensor_mul(T3, T3, WX)
                            nc.vector.tensor_add(B3, q10, T3)
                            nc.vector.tensor_sub(T3, B3, A3)
                            nc.vector.tensor_mul(T3, T3, WY)
                            nc.vector.tensor_add(O3, A3, T3)
                # store supertile output
                for ch in range(C):
                    if sup < 4:
                        r0 = ROWBASES[t0]
                        nc.scalar.dma_start(
                            out_d[ch, r0:r0 + 128, :], outts[ch][0:128, :])
                    else:
                        nc.scalar.dma_start(
                            out_d[ch, 512:528, :], outts[ch][0:16, :])
                        nc.scalar.dma_start(
                            out_d[ch, 524:540, :], outts[ch][16:32, :])

    nc.compile()
    return nc


_cache = {}


def _get_nc():
    if 'nc' not in _cache:
        _cache['nc'] = build()
    return _cache['nc']


def _host_inputs(frame, flow):
    frame = np.ascontiguousarray(frame, dtype=np.float32)
    flow = np.ascontiguousarray(flow, dtype=np.float32)
    fbf = frame.astype(ml_dtypes.bfloat16)

    # wrapped free-order maps: o -> (chunk, r, j); col = 240*chunk + 16*j + q
    oo = np.arange(W)
    kk = oo // XC
    rr = (oo % XC) // 15
    jj = (oo % XC) % 15
    colbase = XC * kk + 16 * jj                      # [W]
    rowbases = np.array(ROWBASES, np.int64)

    xww = (colbase[None, :] + np.arange(16)[:, None]).astype(np.float32)
    xgn = np.arange(W, dtype=np.float32)

    # per-sup partition -> rowtile map
    g_of_p = np.arange(128) // 16                    # [128]
    q_of_p = np.arange(128) % 16

    # table row/col gather indices (global, clipped)
    colidx = np.clip(np.arange(NCHUNK)[:, None] * XC - 6
                     + np.arange(PATW + 1)[None, :], 0, W - 1)  # [8,255]

    in_maps = []
    for core in range(8):
        b, half = divmod(core, 2)
        # ---- tables ----
        # rows[t, s, y] global
        rowidx = np.clip(half * HALF + rowbases[:, None, None] - 6
                         + np.arange(2)[None, :, None]
                         + np.arange(PATR + 1)[None, None, :], 0, H - 1)
        # patches [C, NT, 2, 29, 8, 255]
        patches = fbf[b][:, rowidx[:, :, :, None, None],
                         colidx[None, None, None, :, :]]
        pair = np.stack([patches[..., :PATR, :, :PATW],
                         patches[..., :PATR, :, 1:PATW + 1]], axis=-1)
        # -> [NT, chunk, (s,c), y, x, e] -> [NT, 8, 8, NU*2]
        tbl = np.ascontiguousarray(
            pair.transpose(1, 4, 2, 0, 3, 5, 6)).reshape(NT, NCHUNK, 8,
                                                         NU * 2)

        flr = np.zeros((2, NSUP, 128, W), np.float32)
        flw = np.zeros((2, NSUP, 128, W), np.float32)
        yww = np.zeros((NSUP, 128, W), np.float32)
        ygr = np.zeros((NSUP, 128), np.float32)
        rb6 = np.zeros((NSUP, 128), np.float32)
        fl = flow[b]                                  # [2, 1080, 1920]
        for sup, (t0, ng) in enumerate(SUPS):
            P = 16 * ng
            grow = half * HALF + rowbases[t0 + g_of_p[:P]]   # rowbase
            # row layout: partition p -> row rowbase + q? NO: row layout
            # partition p of supertile = rowbase(t0 + p//16) + p%16
            rowsA = grow + q_of_p[:P]
            flr[:, sup, :P, :] = fl[:, rowsA, :]
            ygr[sup, :P] = rowsA
            rb6[sup, :P] = grow - 6
            # wrapped: partition 16g+q holds pixels (rowbase+rr[o],
            #          colbase[o]+q)
            rowm = grow[:, None] + rr[None, :]               # [P, W]
            colm = colbase[None, :] + q_of_p[:P, None]       # [P, W]
            flw[:, sup, :P, :] = fl[:, rowm, colm]
            yww[sup, :P, :] = rowm.astype(np.float32)
        in_maps.append({
            "tbl_d": tbl,
            "flr_d": flr,
            "flw_d": flw,
            "yww_d": yww,
            "ygr_d": ygr,
            "rb6_d": rb6,
            "xww_d": xww,
            "xgn_d": xgn,
        })
    return in_maps


def run(frame, flow, trace=False, tmpdir=None):
    nc = _get_nc()
    in_maps = _host_inputs(frame, flow)
    res = run_bass_kernel_spmd(nc, in_maps, core_ids=list(range(8)),
                               trace=trace, tmpdir=tmpdir)
    out = np.empty((B, C, H, W), np.float32)
    for core in range(8):
        b, half = divmod(core, 2)
        out[b, :, half * HALF:(half + 1) * HALF, :] = \
            res.results[core]["out_d"].astype(np.float32)
    return out, res


def kernel(frame, flow):
    out, _ = run(np.asarray(frame), np.asarray(flow))
    return out
